# revision 19
# baseline (speedup 1.0000x reference)
"""Trainium2 Bass kernel for a GPT-style transformer block.

Shapes: x [2, 2048, 1024], H=16 heads, D=64, MLP 4x.

Distribution over 8 NeuronCores: data-parallel over batch (cores 0-3 ->
batch 0, cores 4-7 -> batch 1) x sequence-parallel over tokens inside
each batch group. Tokens are stride-4 interleaved (core s of the group
owns global tokens s, s+4, ...), which makes the causal-attention loop
structure identical on every core (required: all 8 cores share one SPMD
program); the rank-dependent causal diagonal masks are shipped as
per-core input data. The only collectives are two 4-rank AllGathers per
group (K first, then V, so the Q projection overlaps them).

Precision: the attention branch (QKV, P@V, Wo) runs in fp8e4m3 with
DoubleRow perf mode (157 TF/s, 2x bf16); its output y is small relative
to the residual stream, so end-to-end error stays ~2.5e-3. QK^T scores
stay bf16 (64-deep contraction gains nothing from DoubleRow). The MLP
stays bf16 (fp8 there costs ~1.7e-2 end-to-end, over budget).
fp8-quantized weights are pre-scaled by 32 host-side (0.02-std weights
would land in e4m3's subnormal range), de-scaled in the psum->SBUF copy.
y is scaled by 16 into fp8 via the softmax-reciprocal trick.

LayerNorm gains/biases are folded into the adjacent weight matrices on
the host, so on-chip LN is a plain standardization with per-partition
(per-token) scalars. Accumulation, LN statistics, softmax and residuals
stay fp32. The softmax denominator comes for free from an extra
ones-column appended to V (row 64 of the PV accumulator), so no
partition-axis reduction is needed.
"""

import os
import sys

for _p in ("/opt/trn_rl_repo", "/root/.axon_site/_ro/trn_rl_repo"):
    if os.path.isdir(_p) and _p not in sys.path:
        sys.path.insert(0, _p)

import numpy as np
import ml_dtypes

import concourse.bass as bass
import concourse.bacc as bacc
import concourse.mybir as mybir
import concourse.tile as tile
from concourse.bass_utils import run_bass_kernel_spmd

F32 = mybir.dt.float32
BF16 = mybir.dt.bfloat16
FP8 = mybir.dt.float8e4
AF = mybir.ActivationFunctionType
OP = mybir.AluOpType
DR = mybir.MatmulPerfMode.DoubleRow
BF16_NP = ml_dtypes.bfloat16
E4_NP = ml_dtypes.float8_e4m3

B, T, C = 2, 2048, 1024
H, D = 16, 64
FF = 4 * C
EPS = 1e-5
P = 128
CH = C // P        # 8 chunks of the channel dim
NPR = CH // 2      # 4 fp8 DoubleRow pairs over the channel dim
NBQ = 4            # local 128-token blocks per core (512 tokens)
NR = 4             # seq ranks per batch group
FCH = FF // P      # 32 chunks of the FF dim
HP = H // 2        # head pairs

WSC = 32.0         # fp8 weight pre-scale
YSC = 16.0         # fp8 attention-out pre-scale

TRACE = False           # set by test harness for profiling
LEVEL = 5               # phase bisection: 1=QKV 2=+AG 3=+attn 4=+Wo 5=full
REPS = 1                # timing: emit the whole block N times, serialized via x
LAST_RESULTS = None     # BassKernelResults of the last run

_CACHE = {}


def _ln_stats(nc, pool, src, tag):
    """Phase 1 of LN: per-token sum and sum-of-squares of `src`."""
    s1 = pool.tile([P, 1], F32, name=f"ln_s1_{tag}", tag=f"ln_s1_{tag}")
    ssq = pool.tile([P, 1], F32, name=f"ln_ssq_{tag}", tag=f"ln_ssq_{tag}")
    sqs = pool.tile([P, C], BF16, name=f"ln_sqs_{tag}", tag="ln_sqs", bufs=1)
    nc.vector.reduce_sum(s1[:, :], src, axis=mybir.AxisListType.X)
    nc.scalar.activation(sqs[:, :], src, AF.Square, accum_out=ssq[:, :])
    return s1, ssq


def _ln_finalize(nc, pool, src, z_bf, s1, ssq, tag):
    """Phase 2 of LN: turn (sum, sumsq) into (x-mean)*rstd -> z_bf."""
    mean = pool.tile([P, 1], F32, name=f"ln_mean_{tag}", tag=f"ln_mean_{tag}")
    var = pool.tile([P, 1], F32, name=f"ln_var_{tag}", tag="ln_var")
    m2 = pool.tile([P, 1], F32, name=f"ln_m2_{tag}", tag="ln_m2")
    std = pool.tile([P, 1], F32, name=f"ln_std_{tag}", tag="ln_std")
    rstd = pool.tile([P, 1], F32, name=f"ln_rstd_{tag}", tag=f"ln_rstd_{tag}")
    nc.vector.tensor_scalar_mul(mean[:, :], s1[:, :], 1.0 / C)
    nc.vector.tensor_mul(m2[:, :], mean[:, :], mean[:, :])
    nc.vector.tensor_scalar(var[:, :], ssq[:, :], 1.0 / C, EPS, OP.mult, OP.add)
    nc.vector.tensor_sub(var[:, :], var[:, :], m2[:, :])
    nc.scalar.activation(std[:, :], var[:, :], AF.Sqrt)
    nc.vector.reciprocal(rstd[:, :], std[:, :])
    nc.vector.tensor_scalar(
        z_bf, src, mean[:, :], rstd[:, :], OP.subtract, OP.mult
    )


def _build(level=5, reps=1, sim=False):
    if (level, reps, sim) in _CACHE:
        return _CACHE[(level, reps, sim)]

    nc = bacc.Bacc(
        "TRN2", target_bir_lowering=False, debug=False,
        num_devices=1 if sim else 8,
    )
    nc._phase_marks = []

    def _mark(name):
        seq = int(nc.get_next_instruction_name().split("-")[1])
        nc._phase_marks.append((name, seq))

    # ---- kernel I/O (per core) ----
    x_in = nc.dram_tensor("x_c", [NBQ, P, C], F32, kind="ExternalInput").ap()
    wqkv_in = nc.dram_tensor(
        "wqkv8", [NPR, P, 2, 3 * C], FP8, kind="ExternalInput"
    ).ap()
    wo_in = nc.dram_tensor("wo8", [NPR, P, 2, C], FP8, kind="ExternalInput").ap()
    wfc_in = nc.dram_tensor("wfc", [CH, P, FF], BF16, kind="ExternalInput").ap()
    wproj_in = nc.dram_tensor("wproj", [FCH, P, C], BF16, kind="ExternalInput").ap()
    bqk_in = nc.dram_tensor("bqk", [2 * CH, P], F32, kind="ExternalInput").ap()
    bv_in = nc.dram_tensor("bv8", [1, 2, C], FP8, kind="ExternalInput").ap()
    bo_in = nc.dram_tensor("bo8", [1, 2, C], FP8, kind="ExternalInput").ap()
    bfc_in = nc.dram_tensor("bfc_r", [FCH, P], F32, kind="ExternalInput").ap()
    bproj_in = nc.dram_tensor("bproj_r", [1, C], BF16, kind="ExternalInput").ap()
    ident_in = nc.dram_tensor("ident", [P, P], BF16, kind="ExternalInput").ap()
    ones_in = nc.dram_tensor("ones_r", [1, P], BF16, kind="ExternalInput").ap()
    ones8_in = nc.dram_tensor("ones8", [1, 2, P], FP8, kind="ExternalInput").ap()
    masks_in = nc.dram_tensor("masks8", [NR, P, 2, P], FP8, kind="ExternalInput").ap()
    out_dram = nc.dram_tensor("out_c", [NBQ, P, C], F32, kind="ExternalOutput").ap()

    KCOLS = CH * 512          # 4096 fp8 cols for K^T in the AG payload
    VCOLS = NBQ * (C + H)     # 4*1040 fp8 cols for aug-V in the AG payload

    with tile.TileContext(nc) as tc:
        dramp = tc.alloc_tile_pool(name="dram", bufs=1, space="DRAM")
        rep_io = [
            dramp.tile([NBQ, P, C], F32, name=f"rep_io_{i}")
            for i in range(reps - 1)
        ]

        for rep in range(reps):
            sfx = f"_{rep}" if reps > 1 else ""
            x_src = x_in if rep == 0 else rep_io[rep - 1]
            out_tgt = out_dram if rep == reps - 1 else rep_io[rep]
            kvin_k = dramp.tile([P, KCOLS], FP8, name=f"kvink{sfx}_a")
            kvin_v = dramp.tile([P, VCOLS], FP8, name=f"kvinv{sfx}_a")
            kvout_k = dramp.tile([NR, P, KCOLS], FP8, name=f"kvoutk{sfx}_a")
            kvout_v = dramp.tile([NR, P, VCOLS], FP8, name=f"kvoutv{sfx}_a")
            # ---------------- persistent SBUF ----------------
            persist = tc.alloc_tile_pool(name=f"persist{sfx}", bufs=1, side="left")
            ident_sb = persist.tile([P, P], BF16, name="ident_sb")
            ones_sb = persist.tile([1, P], BF16, name="ones_sb")
            ones8_sb = persist.tile([1, 2, P], FP8, name="ones8_sb")
            masks_sb = persist.tile([P, NR, 2, P], FP8, name="masks_sb")
            bqk_sb = persist.tile([P, 2 * CH], F32, name="bqk_sb")
            bv_sb = persist.tile([1, 2, C], FP8, name="bv_sb")
            bo_sb = persist.tile([1, 2, C], FP8, name="bo_sb")
            bfc_sb = persist.tile([P, FCH], F32, name="bfc_sb")
            bproj_sb = persist.tile([1, C], BF16, name="bproj_sb")
            qT = persist.tile([P, CH, 512], FP8, name="qT")
            yT = persist.tile([P, CH, 512], FP8, name="yT")

            # K^T gathered from all 4 ranks: [d-part, head-pair chunk, rank, tok]
            attnspan = tc.alloc_tile_pool(name=f"attnspan{sfx}", bufs=1, side="left")
            kfull = attnspan.tile([P, CH, NR, 512], FP8, name="kfull")
            vfull = attnspan.tile([P, NR, NBQ, C + H], FP8, name="vfull")

            _mark("ln1_qkv")
            # ---------------- phase 0: LN1 + QKV + AllGather ----------------
            ph0 = tc.alloc_tile_pool(name=f"ph0{sfx}", bufs=1, side="left")
            ph0w = tc.alloc_tile_pool(name=f"ph0w{sfx}", bufs=2, side="left")
            wqkv_sb = ph0.tile([P, NPR, 2, 3 * C], FP8, name="wqkv_sb")
            hT = ph0.tile([P, CH, 512], FP8, name="hT")
            kTc = ph0.tile([P, CH, 512], FP8, name="kTc")
            vc = ph0.tile([P, NBQ, C + H], FP8, name="vc")

            psQK = tc.alloc_tile_pool(name=f"psQK{sfx}", bufs=4, space="PSUM")
            psV = tc.alloc_tile_pool(name=f"psV{sfx}", bufs=2, space="PSUM")
            psT = tc.alloc_tile_pool(name=f"psT{sfx}", bufs=2, space="PSUM")

            nc.sync.dma_start(ident_sb[:, :], ident_in)
            xts, stats = [], []
            for bq in range(NBQ):
                xt = ph0w.tile([P, C], F32, name="xt", tag="xt", bufs=4)
                nc.gpsimd.dma_start(xt[:, 0:C // 2], x_src[bq][:, 0:C // 2])
                nc.gpsimd.dma_start(xt[:, C // 2:], x_src[bq][:, C // 2:])
                xts.append(xt)
                stats.append(_ln_stats(nc, ph0w, xt[:, :], f"l1_{bq}"))
            for bq in range(NBQ):
                zbf = ph0w.tile([P, C], BF16, name="zbf", tag="zbf")
                _ln_finalize(
                    nc, ph0w, xts[bq][:, :], zbf[:, :], *stats[bq], f"l1_{bq}"
                )
                for cg in range(CH // 4):
                    pt = psT.tile([P, 4, P], BF16, name="pt", tag="pt")
                    for cc in range(4):
                        c = cg * 4 + cc
                        nc.tensor.transpose(
                            pt[:, cc, :], zbf[:, c * P:(c + 1) * P], ident_sb[:, :]
                        )
                    nc.scalar.activation(
                        hT[:, cg * 4:(cg + 1) * 4, bq * P:(bq + 1) * P],
                        pt[:, :, :], AF.Copy,
                    )

            for j in range(NPR):
                nc.gpsimd.dma_start(wqkv_sb[:, j, :, :], wqkv_in[j])
            nc.sync.dma_start(bqk_sb[:, :], bqk_in.rearrange("a p -> p a"))
            nc.sync.dma_start(ones_sb[:, :], ones_in)
            nc.sync.dma_start(ones8_sb[:, :, :], ones8_in)
            for rk in range(NR):
                nc.sync.dma_start(masks_sb[:, rk, :, :], masks_in[rk])
            nc.sync.dma_start(bv_sb[:, :, :], bv_in)
            nc.sync.dma_start(bo_sb[:, :, :], bo_in)
            nc.sync.dma_start(bfc_sb[:, :], bfc_in.rearrange("a p -> p a"))
            nc.sync.dma_start(bproj_sb[:, :], bproj_in)

            # Q^T and K^T: [feat, tok] via lhsT=W pair chunk, rhs=h^T.
            # K^T first so the AllGather can launch while Q^T computes.
            def _qk_tile(ft):
                ps = psQK.tile([P, 512], F32, name="ps_qk", tag="ps_qk")
                for j in range(NPR):
                    nc.tensor.matmul(
                        ps[:, :],
                        wqkv_sb[:, j, :, ft * P:(ft + 1) * P],
                        hT[:, 2 * j:2 * j + 2, :],
                        start=(j == 0),
                        stop=(j == NPR - 1),
                        perf_mode=DR,
                    )
                dest = qT[:, ft, :] if ft < CH else kTc[:, ft - CH, :]
                nc.scalar.activation(
                    dest, ps[:, :], AF.Identity,
                    scale=1.0 / WSC, bias=bqk_sb[:, ft:ft + 1],
                )

            for ft in range(CH, 2 * CH):
                _qk_tile(ft)

            # AllGather K then V across the 4 seq ranks of this batch group
            if level >= 2:
              groups = [[0, 1, 2, 3], [4, 5, 6, 7]]
              for q in range(4):
                  nc.sync.dma_start(
                      kvin_k[:, q * KCOLS // 4:(q + 1) * KCOLS // 4],
                      kTc[:, 2 * q:2 * q + 2, :].rearrange("p c t -> p (c t)"),
                  )
              if sim:
                  for r in range(NR):
                      nc.sync.dma_start(kvout_k[r], kvin_k[:, :])
              else:
                  nc.gpsimd.collective_compute(
                      "AllGather", OP.bypass, replica_groups=groups,
                      ins=[kvin_k.opt()], outs=[kvout_k.opt()],
                  )
            # V in [tok, feat] fp8 layout with a ones column appended per head
            # (col h*65+64) so PV also accumulates the softmax denominator.
            for bq in range(NBQ):
                for fb in range(2):
                    ps = psV.tile([P, 512], F32, name="ps_v", tag="ps_v")
                    for j in range(NPR):
                        nc.tensor.matmul(
                            ps[:, :],
                            hT[:, 2 * j:2 * j + 2, bq * P:(bq + 1) * P],
                            wqkv_sb[:, j, :, 2 * C + fb * 512:2 * C + (fb + 1) * 512],
                            start=(j == 0),
                            stop=False,
                            perf_mode=DR,
                        )
                    nc.tensor.matmul(
                        ps[:, :],
                        ones8_sb[0:1, :, :],
                        bv_sb[0:1, :, fb * 512:(fb + 1) * 512],
                        start=False,
                        stop=True,
                        perf_mode=DR,
                    )
                    dst = vc[:, bq, fb * 8 * 65:(fb + 1) * 8 * 65]
                    dst = dst.rearrange("p (h x) -> p h x", x=65)[:, :, 0:64]
                    nc.scalar.activation(
                        dst, ps.rearrange("p (h x) -> p h x", x=64), AF.Copy,
                        scale=1.0 / WSC,
                    )
            ones_lane = vc.rearrange("p b (h x) -> p b h x", x=65)[:, :, :, 64:65]
            nc.vector.memset(ones_lane, 1.0)

            if level >= 2:
              for q in range(4):
                  nc.sync.dma_start(
                      kvin_v[:, q * VCOLS // 4:(q + 1) * VCOLS // 4],
                      vc[:, q, :],
                  )
              if sim:
                  for r in range(NR):
                      nc.sync.dma_start(kvout_v[r], kvin_v[:, :])
              else:
                  nc.gpsimd.collective_compute(
                      "AllGather", OP.bypass, replica_groups=groups,
                      ins=[kvin_v.opt()], outs=[kvout_v.opt()],
                  )
            for ft in range(CH):
                _qk_tile(ft)
            if level >= 2:
              # interleave K/V readback per rank so attention's first
              # (rk=0) blocks unblock after a quarter of the readback
              for r in range(NR):
                  nc.gpsimd.dma_start(
                      kfull[:, :, r, :],
                      kvout_k[r].rearrange("p (c t) -> p c t", t=512),
                  )
                  nc.gpsimd.dma_start(
                      vfull[:, r, :, :],
                      kvout_v[r].rearrange("p (b f) -> p b f", f=C + H),
                  )

            psT.release()
            psV.release()
            psQK.release()
            ph0w.release()
            ph0.release()

            _mark("attn")
            # ---------------- attention ----------------
            x2pool = tc.alloc_tile_pool(name=f"x2pool{sfx}", bufs=1, side="right")
            x2 = x2pool.tile([P, NBQ, C], F32, name="x2")

            wfcpool = tc.alloc_tile_pool(name=f"wfcpool{sfx}", bufs=1, side="right")
            wfc_sb = wfcpool.tile([P, CH, FF], BF16, name="wfc_sb")

            stats2 = [None] * NBQ
            mw = tc.alloc_tile_pool(name=f"mw{sfx}", bufs=2, side="right")
            wospan = tc.alloc_tile_pool(name=f"wospan{sfx}", bufs=1, side="right")
            wo_sb = wospan.tile([P, NPR, 2, C], FP8, name="wo_sb")

            att = tc.alloc_tile_pool(name=f"att{sfx}", bufs=1, side="right")
            psS = tc.alloc_tile_pool(name=f"psS{sfx}", bufs=2, space="PSUM")
            psY = tc.alloc_tile_pool(name=f"psY{sfx}", bufs=1, space="PSUM")
            psB = tc.alloc_tile_pool(name=f"psB{sfx}", bufs=1, space="PSUM")

            for hp in range(HP if level >= 3 else 0):
                # stream the MLP/out-proj weights in during the Act-bound
                # attention window instead of the DMA-bound QKV phase
                if level >= 5:
                    for q in range(4):
                        nc.gpsimd.dma_start(
                            wfc_sb[:, hp, q * FF // 4:(q + 1) * FF // 4],
                            wfc_in[hp][:, q * FF // 4:(q + 1) * FF // 4],
                        )
                if level >= 4 and hp < NPR:
                    nc.gpsimd.dma_start(wo_sb[:, hp, :, :], wo_in[hp])
                psy = [
                    psY.tile([65, 512], F32, name=f"psy{sub}_{hp}", tag=f"psy{sub}")
                    for sub in range(2)
                ]
                for bk in range(NBQ):
                    qo = bk * P
                    for rk2 in range(NR // 2):
                        # exp'd scores for the rank pair, fp8, PV-ready:
                        # [key, sub, rank-in-pair, query]
                        pbf = att.tile(
                            [P, 2, 2, 512], FP8, name="pbf", tag="pbf", bufs=6
                        )
                        for ri in range(2):
                            rk = 2 * rk2 + ri
                            # both heads of the pair score into one 2-bank
                            # psum tile; one Exp covers both
                            pss = psS.tile([P, 2, 512], F32, name="pss", tag="pss")
                            for sub in range(2):
                                po = sub * 64
                                nc.tensor.matmul(
                                    pss[:, sub, qo:],
                                    kfull[po:po + 64, hp, rk, bk * P:(bk + 1) * P],
                                    qT[po:po + 64, hp, qo:],
                                    start=True,
                                    stop=True,
                                    tile_position=(po, 0),
                                )
                            nc.scalar.activation(
                                pbf[:, :, ri, qo:], pss[:, :, qo:], AF.Exp,
                                scale=1.0 / 8.0,
                            )
                            nc.vector.tensor_mul(
                                pbf[:, :, ri, qo:qo + P], pbf[:, :, ri, qo:qo + P],
                                masks_sb[:, rk, :, :],
                            )
                        for sub in range(2):
                            h = 2 * hp + sub
                            nc.tensor.matmul(
                                psy[sub][:, qo:],
                                vfull[:, 2 * rk2:2 * rk2 + 2, bk, h * 65:(h + 1) * 65],
                                pbf[:, sub, :, qo:],
                                start=(bk == 0 and rk2 == 0),
                                stop=(bk == NBQ - 1 and rk2 == 1),
                                perf_mode=DR,
                                skip_group_check=True,
                            )
                for sub in range(2):
                    po = sub * 64
                    den = att.tile([1, 512], F32, name="den", tag="den", bufs=2)
                    nc.vector.tensor_scalar_mul(
                        den[:, :], psy[sub][64:65, :], 1.0 / YSC
                    )
                    recip = att.tile([1, 512], BF16, name="recip", tag="recip", bufs=2)
                    with nc.allow_low_precision(reason="softmax normalizer"):
                        nc.vector.reciprocal(recip[:, :], den[:, :])
                    psb = psB.tile([64, 512], F32, name="psb", tag="psb")
                    nc.tensor.matmul(
                        psb[:, :],
                        ones_sb[0:1, 0:64],
                        recip[0:1, :],
                        start=True,
                        stop=True,
                    )
                    bcast = att.tile([64, 512], BF16, name="bcast", tag="bcast", bufs=2)
                    nc.vector.tensor_copy(bcast[:, :], psb[:, :])
                    nc.vector.tensor_mul(
                        yT[po:po + 64, hp, :], psy[sub][0:64, :], bcast[:, :]
                    )

            psB.release()
            psY.release()
            psS.release()
            att.release()
            attnspan.release()

            _mark("wo_resid")
            # ---------------- attention out-proj + residual ----------------
            wpool = tc.alloc_tile_pool(name=f"wpool{sfx}", bufs=2, side="right")
            psW = tc.alloc_tile_pool(name=f"psW{sfx}", bufs=3, space="PSUM")
            for bq in range(NBQ if level >= 4 else 0):
                xw = wpool.tile([P, C], F32, name="xw", tag="xw")
                nc.gpsimd.dma_start(xw[:, :], x_src[bq])
                for cb in range(2):
                    ps = psW.tile([P, 512], F32, name="ps_w", tag="ps_w")
                    for j in range(NPR):
                        nc.tensor.matmul(
                            ps[:, :],
                            yT[:, 2 * j:2 * j + 2, bq * P:(bq + 1) * P],
                            wo_sb[:, j, :, cb * 512:(cb + 1) * 512],
                            start=(j == 0),
                            stop=False,
                            perf_mode=DR,
                        )
                    nc.tensor.matmul(
                        ps[:, :],
                        ones8_sb[0:1, :, :],
                        bo_sb[0:1, :, cb * 512:(cb + 1) * 512],
                        start=False,
                        stop=True,
                        perf_mode=DR,
                    )
                    nc.vector.scalar_tensor_tensor(
                        x2[:, bq, cb * 512:(cb + 1) * 512],
                        ps[:, :],
                        1.0 / (WSC * YSC),
                        xw[:, cb * 512:(cb + 1) * 512],
                        OP.mult,
                        OP.add,
                    )
                if level >= 5:
                    stats2[bq] = _ln_stats(nc, mw, x2[:, bq, :], f"l2_{bq}")
            psW.release()
            wpool.release()
            wospan.release()

            _mark("mlp_ln2")
            # ---------------- MLP ----------------
            mpool = tc.alloc_tile_pool(name=f"mpool{sfx}", bufs=1, side="right")
            h2T = mpool.tile([P, CH, 512], BF16, name="h2T")
            gT = mpool.tile([P, FCH, 512], BF16, name="gT")

            psT2 = tc.alloc_tile_pool(name=f"psT2{sfx}", bufs=4, space="PSUM")
            for bq in range(NBQ if level >= 5 else 0):
                z2 = mw.tile([P, C], BF16, name="z2", tag="z2")
                _ln_finalize(
                    nc, mw, x2[:, bq, :], z2[:, :], *stats2[bq], f"l2_{bq}"
                )
                for cg in range(CH // 4):
                    pt2 = psT2.tile([P, 4, P], BF16, name="pt2", tag="pt2")
                    for cc in range(4):
                        c = cg * 4 + cc
                        nc.tensor.transpose(
                            pt2[:, cc, :], z2[:, c * P:(c + 1) * P], ident_sb[:, :]
                        )
                    nc.vector.tensor_copy(
                        h2T[:, cg * 4:(cg + 1) * 4, bq * P:(bq + 1) * P], pt2[:, :, :]
                    )
            psT2.release()

            _mark("mlp_fc")
            psPJ = tc.alloc_tile_pool(name=f"psPJ{sfx}", bufs=1, space="PSUM")
            psFC = tc.alloc_tile_pool(name=f"psFC{sfx}", bufs=2, space="PSUM")
            presA = [
                psPJ.tile([P, 512], F32, name=f"presA_{i}", tag=f"presA_{i}")
                for i in range(4)
            ] if level >= 5 else []
            # FC streams per ft chunk; Proj for blocks 0-1 rides one chunk
            # behind so FC and Proj-A share the tail instead of serializing
            for ft in range(FCH if level >= 5 else 0):
                ps = psFC.tile([P, 512], F32, name="ps_fc", tag="ps_fc")
                for c in range(CH):
                    nc.tensor.matmul(
                        ps[:, :],
                        wfc_sb[:, c, ft * P:(ft + 1) * P],
                        h2T[:, c, :],
                        start=(c == 0),
                        stop=(c == CH - 1),
                    )
                nc.scalar.activation(
                    gT[:, ft, :], ps[:, :], AF.Gelu,
                    bias=bfc_sb[:, ft:ft + 1]
                )
                wp = mw.tile([P, C], BF16, name="wp", tag="wp", bufs=6)
                nc.gpsimd.dma_start(wp[:, :], wproj_in[ft])
                for bq in range(2):
                    for cb in range(2):
                        nc.tensor.matmul(
                            presA[bq * 2 + cb][:, :],
                            gT[:, ft, bq * P:(bq + 1) * P],
                            wp[:, cb * 512:(cb + 1) * 512],
                            start=(ft == 0),
                            stop=False,
                        )
            psFC.release()

            _mark("mlp_proj")

            def _proj_out(bq, pres_i):
                for cb in range(2):
                    nc.tensor.matmul(
                        pres_i[cb][:, :],
                        ones_sb[0:1, 0:P],
                        bproj_sb[0:1, cb * 512:(cb + 1) * 512],
                        start=False,
                        stop=True,
                    )
                for cb in range(2):
                    nc.vector.tensor_add(
                        x2[:, bq, cb * 512:(cb + 1) * 512],
                        pres_i[cb][:, :],
                        x2[:, bq, cb * 512:(cb + 1) * 512],
                    )
                    nc.sync.dma_start(
                        out_tgt[bq][:, cb * 512:(cb + 1) * 512],
                        x2[:, bq, cb * 512:(cb + 1) * 512],
                    )

            if level >= 5:
                _proj_out(0, presA[0:2])
                _proj_out(1, presA[2:4])
            psPJ.release()
            psPJb = tc.alloc_tile_pool(name=f"psPJb{sfx}", bufs=1, space="PSUM")
            presB = [
                psPJb.tile([P, 512], F32, name=f"presB_{i}", tag=f"presB_{i}")
                for i in range(4)
            ] if level >= 5 else []
            for fc in range(FCH if level >= 5 else 0):
                wp2 = mw.tile([P, C], BF16, name="wp2", tag="wp", bufs=6)
                nc.gpsimd.dma_start(wp2[:, :], wproj_in[fc])
                for bq in range(2, 4):
                    for cb in range(2):
                        nc.tensor.matmul(
                            presB[(bq - 2) * 2 + cb][:, :],
                            gT[:, fc, bq * P:(bq + 1) * P],
                            wp2[:, cb * 512:(cb + 1) * 512],
                            start=(fc == 0),
                            stop=False,
                        )
            if level >= 5:
                _proj_out(2, presB[0:2])
                _proj_out(3, presB[2:4])
            if level < 5:
                dummy = mw.tile([P, C], F32, name="dummy")
                nc.vector.memset(dummy[:, :], 0.0)
                for bq in range(NBQ):
                    nc.sync.dma_start(out_tgt[bq], dummy[:, :])
            psPJb.release()
            mpool.release()
            mw.release()
            wfcpool.release()
            x2pool.release()
            persist.release()
        dramp.release()

    if not sim:
        nc.compile()
    _CACHE[(level, reps, sim)] = nc
    return nc


def _q8(a, scale):
    return np.clip(np.asarray(a, np.float32) * scale, -240.0, 240.0).astype(E4_NP)


def prepare_in_maps(inputs):
    """Host-side prep: fold LN, cast/shard weights, build per-core input maps."""
    x = np.asarray(inputs["x"], dtype=np.float32)
    ln1_w = np.asarray(inputs["ln1_w"], dtype=np.float32)
    ln1_b = np.asarray(inputs["ln1_b"], dtype=np.float32)
    Wqkv = np.asarray(inputs["Wqkv"], dtype=np.float32)
    bqkv = np.asarray(inputs["bqkv"], dtype=np.float32)
    Wo = np.asarray(inputs["Wo"], dtype=np.float32)
    bo = np.asarray(inputs["bo"], dtype=np.float32)
    ln2_w = np.asarray(inputs["ln2_w"], dtype=np.float32)
    ln2_b = np.asarray(inputs["ln2_b"], dtype=np.float32)
    Wfc = np.asarray(inputs["Wfc"], dtype=np.float32)
    bfc = np.asarray(inputs["bfc"], dtype=np.float32)
    Wproj = np.asarray(inputs["Wproj"], dtype=np.float32)
    bproj = np.asarray(inputs["bproj"], dtype=np.float32)

    # Fold LN affine params into the downstream matmuls.
    Wqkv_f = ln1_w[:, None] * Wqkv
    bqkv_f = bqkv + ln1_b @ Wqkv
    Wfc_f = ln2_w[:, None] * Wfc
    bfc_f = bfc + ln2_b @ Wfc

    # fp8 pair-major weights: [pair, P, 2, out], pre-scaled by WSC
    wqkv8_h = np.ascontiguousarray(
        _q8(Wqkv_f, WSC).reshape(NPR, 2, P, 3 * C).transpose(0, 2, 1, 3)
    )
    wo8_h = np.ascontiguousarray(
        _q8(Wo, WSC).reshape(NPR, 2, P, C).transpose(0, 2, 1, 3)
    )
    wfc_h = np.ascontiguousarray(Wfc_f.astype(BF16_NP).reshape(CH, P, FF))
    wproj_h = np.ascontiguousarray(Wproj.astype(BF16_NP).reshape(FCH, P, C))
    bqk_h = np.ascontiguousarray(bqkv_f[: 2 * C].reshape(2 * CH, P))
    bv8_h = np.zeros((1, 2, C), E4_NP)
    bv8_h[0, 0] = _q8(bqkv_f[2 * C:], WSC)
    bo8_h = np.zeros((1, 2, C), E4_NP)
    bo8_h[0, 0] = _q8(bo, WSC * YSC)
    bfc_h = np.ascontiguousarray(bfc_f.reshape(FCH, P))
    bproj_h = bproj.astype(BF16_NP).reshape(1, C)
    ident_h = np.eye(P, dtype=BF16_NP)
    ones_h = np.ones((1, P), BF16_NP)
    ones8_h = np.zeros((1, 2, P), E4_NP)
    ones8_h[0, 0] = 1.0
    kk = np.arange(P)[:, None]
    qq = np.arange(P)[None, :]
    tri_incl = (kk <= qq).astype(E4_NP)
    tri_strict = (kk < qq).astype(E4_NP)

    in_maps = []
    for core in range(8):
        b, s = divmod(core, 4)
        x_c = np.ascontiguousarray(x[b, s::4, :]).reshape(NBQ, P, C)
        masks_h = np.stack(
            [tri_incl if rk <= s else tri_strict for rk in range(NR)]
        )
        masks_h = np.repeat(masks_h[:, :, None, :], 2, axis=2)
        in_maps.append(
            {
                "x_c": x_c,
                "wqkv8": wqkv8_h,
                "wo8": wo8_h,
                "wfc": wfc_h,
                "wproj": wproj_h,
                "bqk": bqk_h,
                "bv8": bv8_h,
                "bo8": bo8_h,
                "bfc_r": bfc_h,
                "bproj_r": bproj_h,
                "ident": ident_h,
                "ones_r": ones_h,
                "ones8": ones8_h,
                "masks8": np.ascontiguousarray(masks_h),
            }
        )

    return in_maps


def assemble_output(results):
    out = np.empty((B, T, C), np.float32)
    for core in range(8):
        b, s = divmod(core, 4)
        out[b, s::4, :] = results[core]["out_c"].reshape(NR * P, C)
    return out


def kernel(**inputs):
    global LAST_RESULTS
    in_maps = prepare_in_maps(inputs)
    nc = _build(LEVEL, REPS)
    res = run_bass_kernel_spmd(
        nc, in_maps, core_ids=list(range(8)), trace=TRACE
    )
    LAST_RESULTS = res
    return assemble_output(res.results)
